# revision 2
# baseline (speedup 1.0000x reference)
"""GNN message passing (src_mul_edge + segment_sum) on 8 Trainium2 cores. v5.

out[n] = sum_{e : dst[e]==n} e_att[e] * src_emb[src[e]]

Non-transpose pair-token gather (baseline-proven DMA mode, tight layout):
  * src_emb rows cast to fp16 (unpadded); consecutive row PAIRS form 256-byte
    tokens in DRAM ([25088, 128] fp16). Token ids fit int16 -> SINGLE index
    window, no lo/hi split. Edge with src row r uses half r%2 of token r//2;
    the unused half is zeroed by the att table.
  * Nodes sorted by total degree, dealt 128 at a time into tiles (lane =
    node); tile span S = max degree in the 128-node window (~1% padding).
    Slot (lane, s) = flat gather position s*128+lane; per-edge descriptors.
  * dma_gather(transpose=False) from DRAM: msg[lane, s, 0:128] = token fp16.
  * att3 [128, S_total, 2] fp16 (host-built, half-selected att or 0) loaded
    once; one broadcast multiply per chunk ([128, S, 2] -> [128, S, 2, 64]);
    strided tensor_reduce over (2S) per tile -> out [128 nodes, 64] fp32,
    one DMA per tile straight to DRAM.
  * Within a node, edges sorted by token id (HBM row locality).
"""

import numpy as np

N_SRC = 50000
N_DST = 50000
D = 64
N_CORES = 8
P = 128
NPAIR = 25088
GCHUNK = 4096             # slots per gather instruction (mult of 128)

_cache: dict = {}

TRACE = False
TRACE_DIR = None
LAST_EXEC_NS = None


def _wrap_idx(idx_flat):
    w = idx_flat.reshape(-1, 16).T
    return np.tile(w, (8, 1))


def _plan(dst_idx, tok, half, att):
    """Single layout over all edges. Returns schedule + per-core arrays."""
    deg = np.bincount(dst_idx, minlength=N_DST)
    nz = np.flatnonzero(deg)
    order = nz[np.argsort(deg[nz], kind="stable")]
    n_nz = len(order)
    npad = (-n_nz) % (P * N_CORES)
    node_seq = np.concatenate([np.full(npad, -1, dtype=np.int64), order])
    n_tiles = len(node_seq) // (P * N_CORES)
    # tile t, core c, lane l -> node_seq[((t*NC)+c)*P + l]
    node_at = node_seq.reshape(n_tiles, N_CORES, P)
    degs = np.where(node_at >= 0, deg[np.clip(node_at, 0, None)], 0)
    S = degs.max(axis=(1, 2)).astype(np.int64)  # per-tile span
    S = np.maximum(S, 1)

    csum = np.concatenate([[0], np.cumsum(S)])
    C = int(csum[-1])  # total slots per lane... columns = slots per lane

    # chunks of tiles with <= GCHUNK slots (slots = S*128 per tile)
    chunks = []  # (tile0, ntiles, [S...])
    t0 = 0
    while t0 < n_tiles:
        t1 = t0
        acc = 0
        while t1 < n_tiles and (acc + S[t1]) * P <= GCHUNK:
            acc += S[t1]
            t1 += 1
        t1 = max(t1, t0 + 1)
        chunks.append((t0, t1 - t0, tuple(int(x) for x in S[t0:t1])))
        t0 = t1

    # per-edge placement
    ord_of = np.full(N_DST, -1, dtype=np.int64)
    core_of = np.full(N_DST, -1, dtype=np.int64)
    lane_of = np.full(N_DST, -1, dtype=np.int64)
    valid = node_at >= 0
    t_idx = np.broadcast_to(np.arange(n_tiles)[:, None, None], node_at.shape)
    c_idx = np.broadcast_to(np.arange(N_CORES)[None, :, None], node_at.shape)
    l_idx = np.broadcast_to(np.arange(P)[None, None, :], node_at.shape)
    ord_of[node_at[valid]] = t_idx[valid]
    core_of[node_at[valid]] = c_idx[valid]
    lane_of[node_at[valid]] = l_idx[valid]

    eorder = np.lexsort((tok, dst_idx))
    d_sorted = dst_idx[eorder]
    starts = np.concatenate([[0], np.cumsum(deg)])
    rank_e = np.arange(len(dst_idx)) - starts[d_sorted]

    t_e = ord_of[d_sorted]
    c_e = core_of[d_sorted]
    l_e = lane_of[d_sorted]
    s_e = csum[t_e] + rank_e  # slot row within the lane

    idx3 = np.zeros((N_CORES, C, P), dtype=np.int16)
    att4 = np.zeros((N_CORES, C, P, 2), dtype=np.float16)
    idx3[c_e, s_e, l_e] = tok[eorder]
    att4[c_e, s_e, l_e, half[eorder]] = att[eorder]

    return {
        "sched": tuple(chunks),
        "S": tuple(int(x) for x in S),
        "csum": csum,
        "n_tiles": n_tiles,
        "C": C,
        "idx3": idx3,
        "att4": att4,
        "node_at": node_at,
    }


def _build_nc2(sched, csum_list, n_tiles, C):
    import concourse.bacc as bacc
    import concourse.mybir as mybir
    from concourse.tile import TileContext
    from concourse.library_config import mlp

    nc = bacc.Bacc(
        "TRN2", target_bir_lowering=False, debug=False, num_swdge_queues=4
    )
    embP = nc.dram_tensor("embP", [NPAIR, P], mybir.dt.float16, kind="ExternalInput")
    idxT = nc.dram_tensor("idxT", [P, C * P // 16], mybir.dt.int16, kind="ExternalInput")
    attT = nc.dram_tensor("attT", [P, C * 2], mybir.dt.float16, kind="ExternalInput")
    out = nc.dram_tensor("out", [n_tiles * P, D], mybir.dt.float32, kind="ExternalOutput")

    with TileContext(nc) as tc:
        nc.gpsimd.load_library(mlp)
        with (
            tc.tile_pool(name="tbl", bufs=1) as tbl,
            tc.tile_pool(name="msg", bufs=6) as msgp,
            tc.tile_pool(name="stg", bufs=3) as stgp,
        ):
            idx_sb = tbl.tile([P, C * P // 16], mybir.dt.int16, tag="idx")
            att_sb = tbl.tile([P, C * 2], mybir.dt.float16, tag="att")
            nc.sync.dma_start(idx_sb[:], idxT[:])
            nc.scalar.dma_start(att_sb[:], attT[:])

            smax = max(sum(Ss) for _, _, Ss in sched)
            qrot = 0
            for t0, ntl, Ss in sched:
                ssum = sum(Ss)
                nidx = ssum * P
                col0 = csum_list[t0]  # slot offset
                msg = msgp.tile([P, smax, P], mybir.dt.float16, tag="m")
                nc.gpsimd.dma_gather(
                    msg[:, :ssum, :], embP[:, :],
                    idx_sb[:, col0 * P // 16 : (col0 + ssum) * P // 16],
                    nidx, nidx, P,
                    transpose=False, single_packet=False, queue_num=qrot % 4,
                )
                qrot += 1
                att_b = (
                    att_sb[:, col0 * 2 : (col0 + ssum) * 2]
                    .rearrange("p (s h) -> p s h", h=2)
                    .unsqueeze(3)
                    .broadcast_to([P, ssum, 2, D])
                )
                nc.vector.tensor_tensor(
                    msg[:, :ssum, :].rearrange("p s (h d) -> p s h d", h=2),
                    msg[:, :ssum, :].rearrange("p s (h d) -> p s h d", h=2),
                    att_b,
                    mybir.AluOpType.mult,
                )
                so = 0
                for k, S in enumerate(Ss):
                    stage = stgp.tile([P, D], mybir.dt.float32, tag="st")
                    nc.vector.tensor_reduce(
                        stage[:, :],
                        msg[:, so : so + S, :]
                        .rearrange("p s (h d) -> p d (s h)", h=2),
                        axis=mybir.AxisListType.X, op=mybir.AluOpType.add,
                    )
                    nc.sync.dma_start(
                        out[(t0 + k) * P : (t0 + k + 1) * P, :], stage[:, :]
                    )
                    so += S
    nc.compile()
    return nc


def plan_and_build(src_idx, dst_idx, e_att):
    src_idx = np.asarray(src_idx, dtype=np.int64)
    dst_idx = np.asarray(dst_idx, dtype=np.int64)
    att_flat = np.asarray(e_att, dtype=np.float16).reshape(-1)
    tok = (src_idx // 2).astype(np.int16)
    half = (src_idx & 1).astype(np.int64)
    return _plan(dst_idx, tok, half, att_flat)


def kernel(src_emb, e_att, src_idx, dst_idx):
    from concourse.bass_utils import run_bass_kernel_spmd

    src_emb = np.asarray(src_emb, dtype=np.float32)
    pl = plan_and_build(src_idx, dst_idx, e_att)

    key = (pl["sched"],)
    if key not in _cache:
        _cache.clear()
        _cache[key] = _build_nc2(pl["sched"], pl["csum"], pl["n_tiles"], pl["C"])
    nc = _cache[key]

    embP = np.zeros((NPAIR * 2, D), dtype=np.float16)
    embP[:N_SRC] = src_emb.astype(np.float16)
    embP = np.ascontiguousarray(embP.reshape(NPAIR, P))

    in_maps = []
    for c in range(N_CORES):
        idx_flat = pl["idx3"][c].reshape(-1)  # [C*P] in (slot, lane) order
        att2 = np.ascontiguousarray(
            pl["att4"][c].transpose(1, 0, 2).reshape(P, pl["C"] * 2)
        )  # [lane, slot*2]
        in_maps.append(
            {
                "embP": embP,
                "idxT": np.ascontiguousarray(_wrap_idx(idx_flat)),
                "attT": att2,
            }
        )
    kwargs = {}
    if TRACE:
        kwargs = {"trace": True, "tmpdir": TRACE_DIR}
    res = run_bass_kernel_spmd(nc, in_maps, core_ids=list(range(N_CORES)), **kwargs)
    global LAST_EXEC_NS
    LAST_EXEC_NS = res.exec_time_ns

    out_full = np.zeros((N_DST, D), dtype=np.float32)
    node_at = pl["node_at"]  # [n_tiles, 8, P]
    for c in range(N_CORES):
        ids = node_at[:, c, :].reshape(-1)
        valid = ids >= 0
        out_full[ids[valid]] = res.results[c]["out"][valid]
    return out_full


# revision 3
# speedup vs baseline: 1.1517x; 1.1517x over previous
"""GNN message passing (src_mul_edge + segment_sum) on 8 Trainium2 cores. v5.

out[n] = sum_{e : dst[e]==n} e_att[e] * src_emb[src[e]]

Non-transpose pair-token gather (baseline-proven DMA mode, tight layout):
  * src_emb rows cast to fp16 (unpadded); consecutive row PAIRS form 256-byte
    tokens in DRAM ([25088, 128] fp16). Token ids fit int16 -> SINGLE index
    window, no lo/hi split. Edge with src row r uses half r%2 of token r//2;
    the unused half is zeroed by the att table.
  * Nodes sorted by total degree, dealt 128 at a time into tiles (lane =
    node); tile span S = max degree in the 128-node window (~1% padding).
    Slot (lane, s) = flat gather position s*128+lane; per-edge descriptors.
  * dma_gather(transpose=False) from DRAM: msg[lane, s, 0:128] = token fp16.
  * att3 [128, S_total, 2] fp16 (host-built, half-selected att or 0) loaded
    once; one broadcast multiply per chunk ([128, S, 2] -> [128, S, 2, 64]);
    strided tensor_reduce over (2S) per tile -> out [128 nodes, 64] fp32,
    one DMA per tile straight to DRAM.
  * Within a node, edges sorted by token id (HBM row locality).
"""

import numpy as np

N_SRC = 50000
N_DST = 50000
D = 64
N_CORES = 8
P = 128
NPAIR = 25088
GCHUNK = 4096             # slots per gather instruction (mult of 128)

_cache: dict = {}

TRACE = False
TRACE_DIR = None
LAST_EXEC_NS = None


def _wrap_idx(idx_flat):
    w = idx_flat.reshape(-1, 16).T
    return np.tile(w, (8, 1))


def _plan(dst_idx, tok, half, att):
    """Single layout over all edges. Returns schedule + per-core arrays."""
    deg = np.bincount(dst_idx, minlength=N_DST)
    nz = np.flatnonzero(deg)
    order = nz[np.argsort(deg[nz], kind="stable")]
    n_nz = len(order)
    npad = (-n_nz) % (P * N_CORES)
    node_seq = np.concatenate([np.full(npad, -1, dtype=np.int64), order])
    n_tiles = len(node_seq) // (P * N_CORES)
    # tile t, core c, lane l -> node_seq[((t*NC)+c)*P + l]
    node_at = node_seq.reshape(n_tiles, N_CORES, P)
    degs = np.where(node_at >= 0, deg[np.clip(node_at, 0, None)], 0)
    S = degs.max(axis=(1, 2)).astype(np.int64)  # per-tile span
    S = np.maximum(S, 1)

    csum = np.concatenate([[0], np.cumsum(S)])
    C = int(csum[-1])  # total slots per lane... columns = slots per lane

    # chunks of tiles with <= GCHUNK slots (slots = S*128 per tile)
    chunks = []  # (tile0, ntiles, [S...])
    t0 = 0
    while t0 < n_tiles:
        t1 = t0
        acc = 0
        while t1 < n_tiles and (acc + S[t1]) * P <= GCHUNK:
            acc += S[t1]
            t1 += 1
        t1 = max(t1, t0 + 1)
        chunks.append((t0, t1 - t0, tuple(int(x) for x in S[t0:t1])))
        t0 = t1

    # per-edge placement
    ord_of = np.full(N_DST, -1, dtype=np.int64)
    core_of = np.full(N_DST, -1, dtype=np.int64)
    lane_of = np.full(N_DST, -1, dtype=np.int64)
    valid = node_at >= 0
    t_idx = np.broadcast_to(np.arange(n_tiles)[:, None, None], node_at.shape)
    c_idx = np.broadcast_to(np.arange(N_CORES)[None, :, None], node_at.shape)
    l_idx = np.broadcast_to(np.arange(P)[None, None, :], node_at.shape)
    ord_of[node_at[valid]] = t_idx[valid]
    core_of[node_at[valid]] = c_idx[valid]
    lane_of[node_at[valid]] = l_idx[valid]

    eorder = np.lexsort((tok, dst_idx))
    d_sorted = dst_idx[eorder]
    starts = np.concatenate([[0], np.cumsum(deg)])
    rank_e = np.arange(len(dst_idx)) - starts[d_sorted]

    t_e = ord_of[d_sorted]
    c_e = core_of[d_sorted]
    l_e = lane_of[d_sorted]
    s_e = csum[t_e] + rank_e  # slot row within the lane

    idx3 = np.zeros((N_CORES, C, P), dtype=np.int16)
    att4 = np.zeros((N_CORES, C, P, 2), dtype=np.float16)
    idx3[c_e, s_e, l_e] = tok[eorder]
    att4[c_e, s_e, l_e, half[eorder]] = att[eorder]

    return {
        "sched": tuple(chunks),
        "S": tuple(int(x) for x in S),
        "csum": csum,
        "n_tiles": n_tiles,
        "C": C,
        "idx3": idx3,
        "att4": att4,
        "node_at": node_at,
    }


def _build_nc2(sched, csum_list, n_tiles, C):
    import concourse.bacc as bacc
    import concourse.mybir as mybir
    from concourse.tile import TileContext
    from concourse.library_config import mlp

    nc = bacc.Bacc(
        "TRN2", target_bir_lowering=False, debug=False, num_swdge_queues=4
    )
    embP = nc.dram_tensor("embP", [NPAIR, P], mybir.dt.float16, kind="ExternalInput")
    idxT = nc.dram_tensor("idxT", [P, C * P // 16], mybir.dt.int16, kind="ExternalInput")
    attT = nc.dram_tensor("attT", [P, C * 2], mybir.dt.float16, kind="ExternalInput")
    out = nc.dram_tensor("out", [n_tiles * P, D], mybir.dt.float32, kind="ExternalOutput")

    with TileContext(nc) as tc:
        nc.gpsimd.load_library(mlp)
        with (
            tc.tile_pool(name="tbl", bufs=1) as tbl,
            tc.tile_pool(name="msg", bufs=6) as msgp,
            tc.tile_pool(name="stg", bufs=3) as stgp,
        ):
            idx_sb = tbl.tile([P, C * P // 16], mybir.dt.int16, tag="idx")
            att_sb = tbl.tile([P, C * 2], mybir.dt.float16, tag="att")
            nc.sync.dma_start(idx_sb[:], idxT[:])
            nc.scalar.dma_start(att_sb[:], attT[:])

            smax = max(sum(Ss) for _, _, Ss in sched)
            qrot = 0
            for t0, ntl, Ss in sched:
                ssum = sum(Ss)
                nidx = ssum * P
                col0 = csum_list[t0]  # slot offset
                msg = msgp.tile([P, smax, P], mybir.dt.float16, tag="m")
                nc.gpsimd.dma_gather(
                    msg[:, :ssum, :], embP[:, :],
                    idx_sb[:, col0 * P // 16 : (col0 + ssum) * P // 16],
                    nidx, nidx, P,
                    transpose=False, single_packet=False, queue_num=qrot % 4,
                )
                qrot += 1
                att_b = (
                    att_sb[:, col0 * 2 : (col0 + ssum) * 2]
                    .rearrange("p (s h) -> p s h", h=2)
                    .unsqueeze(3)
                    .broadcast_to([P, ssum, 2, D])
                )
                nc.vector.tensor_tensor(
                    msg[:, :ssum, :].rearrange("p s (h d) -> p s h d", h=2),
                    msg[:, :ssum, :].rearrange("p s (h d) -> p s h d", h=2),
                    att_b,
                    mybir.AluOpType.mult,
                )
                so = 0
                for k, S in enumerate(Ss):
                    # pairwise tree over slots with contiguous fp16 adds
                    n = S
                    while n > 1:
                        h_n = n // 2
                        nc.vector.tensor_tensor(
                            msg[:, so : so + h_n, :],
                            msg[:, so : so + h_n, :],
                            msg[:, so + n - h_n : so + n, :],
                            mybir.AluOpType.add,
                        )
                        n = n - h_n
                    stage = stgp.tile([P, D], mybir.dt.float32, tag="st")
                    nc.vector.tensor_tensor(
                        stage[:, :].unsqueeze(1),
                        msg[:, so : so + 1, 0:D],
                        msg[:, so : so + 1, D : 2 * D],
                        mybir.AluOpType.add,
                    )
                    nc.sync.dma_start(
                        out[(t0 + k) * P : (t0 + k + 1) * P, :], stage[:, :]
                    )
                    so += S
    nc.compile()
    return nc


def plan_and_build(src_idx, dst_idx, e_att):
    src_idx = np.asarray(src_idx, dtype=np.int64)
    dst_idx = np.asarray(dst_idx, dtype=np.int64)
    att_flat = np.asarray(e_att, dtype=np.float16).reshape(-1)
    tok = (src_idx // 2).astype(np.int16)
    half = (src_idx & 1).astype(np.int64)
    return _plan(dst_idx, tok, half, att_flat)


def kernel(src_emb, e_att, src_idx, dst_idx):
    from concourse.bass_utils import run_bass_kernel_spmd

    src_emb = np.asarray(src_emb, dtype=np.float32)
    pl = plan_and_build(src_idx, dst_idx, e_att)

    key = (pl["sched"],)
    if key not in _cache:
        _cache.clear()
        _cache[key] = _build_nc2(pl["sched"], pl["csum"], pl["n_tiles"], pl["C"])
    nc = _cache[key]

    embP = np.zeros((NPAIR * 2, D), dtype=np.float16)
    embP[:N_SRC] = src_emb.astype(np.float16)
    embP = np.ascontiguousarray(embP.reshape(NPAIR, P))

    in_maps = []
    for c in range(N_CORES):
        idx_flat = pl["idx3"][c].reshape(-1)  # [C*P] in (slot, lane) order
        att2 = np.ascontiguousarray(
            pl["att4"][c].transpose(1, 0, 2).reshape(P, pl["C"] * 2)
        )  # [lane, slot*2]
        in_maps.append(
            {
                "embP": embP,
                "idxT": np.ascontiguousarray(_wrap_idx(idx_flat)),
                "attT": att2,
            }
        )
    kwargs = {}
    if TRACE:
        kwargs = {"trace": True, "tmpdir": TRACE_DIR}
    res = run_bass_kernel_spmd(nc, in_maps, core_ids=list(range(N_CORES)), **kwargs)
    global LAST_EXEC_NS
    LAST_EXEC_NS = res.exec_time_ns

    out_full = np.zeros((N_DST, D), dtype=np.float32)
    node_at = pl["node_at"]  # [n_tiles, 8, P]
    for c in range(N_CORES):
        ids = node_at[:, c, :].reshape(-1)
        valid = ids >= 0
        out_full[ids[valid]] = res.results[c]["out"][valid]
    return out_full


# revision 4
# speedup vs baseline: 1.1854x; 1.0292x over previous
"""GNN message passing (src_mul_edge + segment_sum) on 8 Trainium2 cores. v5.

out[n] = sum_{e : dst[e]==n} e_att[e] * src_emb[src[e]]

Non-transpose pair-token gather (baseline-proven DMA mode, tight layout):
  * src_emb rows cast to fp16 (unpadded); consecutive row PAIRS form 256-byte
    tokens in DRAM ([25088, 128] fp16). Token ids fit int16 -> SINGLE index
    window, no lo/hi split. Edge with src row r uses half r%2 of token r//2;
    the unused half is zeroed by the att table.
  * Nodes sorted by total degree, dealt 128 at a time into tiles (lane =
    node); tile span S = max degree in the 128-node window (~1% padding).
    Slot (lane, s) = flat gather position s*128+lane; per-edge descriptors.
  * dma_gather(transpose=False) from DRAM: msg[lane, s, 0:128] = token fp16.
  * att3 [128, S_total, 2] fp16 (host-built, half-selected att or 0) loaded
    once; one broadcast multiply per chunk ([128, S, 2] -> [128, S, 2, 64]);
    strided tensor_reduce over (2S) per tile -> out [128 nodes, 64] fp32,
    one DMA per tile straight to DRAM.
  * Within a node, edges sorted by token id (HBM row locality).
"""

import numpy as np

N_SRC = 50000
N_DST = 50000
D = 64
N_CORES = 8
P = 128
NPAIR = 25088
GCHUNK = 4096             # slots per gather instruction (mult of 128)

_cache: dict = {}

TRACE = False
TRACE_DIR = None
LAST_EXEC_NS = None


def _wrap_idx(idx_flat):
    w = idx_flat.reshape(-1, 16).T
    return np.tile(w, (8, 1))


def _plan(dst_idx, tok, half, att):
    """Single layout over all edges. Returns schedule + per-core arrays."""
    deg = np.bincount(dst_idx, minlength=N_DST)
    nz = np.flatnonzero(deg)
    order = nz[np.argsort(deg[nz], kind="stable")]
    n_nz = len(order)
    npad = (-n_nz) % (P * N_CORES)
    node_seq = np.concatenate([np.full(npad, -1, dtype=np.int64), order])
    n_tiles = len(node_seq) // (P * N_CORES)
    # tile t, core c, lane l -> node_seq[((t*NC)+c)*P + l]
    node_at = node_seq.reshape(n_tiles, N_CORES, P)
    degs = np.where(node_at >= 0, deg[np.clip(node_at, 0, None)], 0)
    S = degs.max(axis=(1, 2)).astype(np.int64)  # per-tile span
    S = np.maximum(S, 1)

    csum = np.concatenate([[0], np.cumsum(S)])
    C = int(csum[-1])  # total slots per lane... columns = slots per lane

    # chunks of tiles with <= GCHUNK slots (slots = S*128 per tile)
    chunks = []  # (tile0, ntiles, [S...])
    t0 = 0
    while t0 < n_tiles:
        t1 = t0
        acc = 0
        while t1 < n_tiles and (acc + S[t1]) * P <= GCHUNK:
            acc += S[t1]
            t1 += 1
        t1 = max(t1, t0 + 1)
        chunks.append((t0, t1 - t0, tuple(int(x) for x in S[t0:t1])))
        t0 = t1

    # per-edge placement
    ord_of = np.full(N_DST, -1, dtype=np.int64)
    core_of = np.full(N_DST, -1, dtype=np.int64)
    lane_of = np.full(N_DST, -1, dtype=np.int64)
    valid = node_at >= 0
    t_idx = np.broadcast_to(np.arange(n_tiles)[:, None, None], node_at.shape)
    c_idx = np.broadcast_to(np.arange(N_CORES)[None, :, None], node_at.shape)
    l_idx = np.broadcast_to(np.arange(P)[None, None, :], node_at.shape)
    ord_of[node_at[valid]] = t_idx[valid]
    core_of[node_at[valid]] = c_idx[valid]
    lane_of[node_at[valid]] = l_idx[valid]

    eorder = np.lexsort((tok, dst_idx))
    d_sorted = dst_idx[eorder]
    starts = np.concatenate([[0], np.cumsum(deg)])
    rank_e = np.arange(len(dst_idx)) - starts[d_sorted]

    t_e = ord_of[d_sorted]
    c_e = core_of[d_sorted]
    l_e = lane_of[d_sorted]
    s_e = csum[t_e] + rank_e  # slot row within the lane

    idx3 = np.zeros((N_CORES, C, P), dtype=np.int16)
    att4 = np.zeros((N_CORES, C, P, 2), dtype=np.float16)
    idx3[c_e, s_e, l_e] = tok[eorder]
    att4[c_e, s_e, l_e, half[eorder]] = att[eorder]

    return {
        "sched": tuple(chunks),
        "S": tuple(int(x) for x in S),
        "csum": csum,
        "n_tiles": n_tiles,
        "C": C,
        "idx3": idx3,
        "att4": att4,
        "node_at": node_at,
    }


def _build_nc2(sched, csum_list, n_tiles, C):
    import concourse.bacc as bacc
    import concourse.mybir as mybir
    from concourse.tile import TileContext
    from concourse.library_config import mlp

    nc = bacc.Bacc(
        "TRN2", target_bir_lowering=False, debug=False, num_swdge_queues=4
    )
    embP = nc.dram_tensor("embP", [NPAIR, P], mybir.dt.float16, kind="ExternalInput")
    idxT = nc.dram_tensor("idxT", [P, C * P // 16], mybir.dt.int16, kind="ExternalInput")
    attX = nc.dram_tensor("attX", [P, C * P], mybir.dt.float16, kind="ExternalInput")
    out = nc.dram_tensor("out", [n_tiles * P, D], mybir.dt.float32, kind="ExternalOutput")

    with TileContext(nc) as tc:
        nc.gpsimd.load_library(mlp)
        with (
            tc.tile_pool(name="tbl", bufs=1) as tbl,
            tc.tile_pool(name="msg", bufs=8) as msgp,
            tc.tile_pool(name="attx", bufs=3) as attxp,
            tc.tile_pool(name="stg", bufs=3) as stgp,
        ):
            idx_sb = tbl.tile([P, C * P // 16], mybir.dt.int16, tag="idx")
            nc.sync.dma_start(idx_sb[:], idxT[:])

            smax = max(sum(Ss) for _, _, Ss in sched)
            # greedy queue balance by descriptor count
            qload = [0, 0, 0, 0]
            for t0, ntl, Ss in sched:
                ssum = sum(Ss)
                nidx = ssum * P
                col0 = csum_list[t0]  # slot offset
                q = min(range(4), key=lambda i: qload[i])
                qload[q] += nidx
                msg = msgp.tile([P, smax, P], mybir.dt.float16, tag="m")
                nc.gpsimd.dma_gather(
                    msg[:, :ssum, :], embP[:, :],
                    idx_sb[:, col0 * P // 16 : (col0 + ssum) * P // 16],
                    nidx, nidx, P,
                    transpose=False, single_packet=False, queue_num=q,
                )
                att_x = attxp.tile([P, smax, P], mybir.dt.float16, tag="ax")
                nc.scalar.dma_start(
                    att_x[:, :ssum, :],
                    attX[:, col0 * P : (col0 + ssum) * P]
                    .rearrange("p (s d) -> p s d", d=P),
                )
                nc.vector.tensor_tensor(
                    msg[:, :ssum, :],
                    msg[:, :ssum, :],
                    att_x[:, :ssum, :],
                    mybir.AluOpType.mult,
                )
                so = 0
                for k, S in enumerate(Ss):
                    # pairwise tree over slots with contiguous fp16 adds
                    n = S
                    while n > 1:
                        h_n = n // 2
                        nc.vector.tensor_tensor(
                            msg[:, so : so + h_n, :],
                            msg[:, so : so + h_n, :],
                            msg[:, so + n - h_n : so + n, :],
                            mybir.AluOpType.add,
                        )
                        n = n - h_n
                    stage = stgp.tile([P, D], mybir.dt.float32, tag="st")
                    nc.vector.tensor_tensor(
                        stage[:, :].unsqueeze(1),
                        msg[:, so : so + 1, 0:D],
                        msg[:, so : so + 1, D : 2 * D],
                        mybir.AluOpType.add,
                    )
                    nc.sync.dma_start(
                        out[(t0 + k) * P : (t0 + k + 1) * P, :], stage[:, :]
                    )
                    so += S
    nc.compile()
    return nc


def plan_and_build(src_idx, dst_idx, e_att):
    src_idx = np.asarray(src_idx, dtype=np.int64)
    dst_idx = np.asarray(dst_idx, dtype=np.int64)
    att_flat = np.asarray(e_att, dtype=np.float16).reshape(-1)
    tok = (src_idx // 2).astype(np.int16)
    half = (src_idx & 1).astype(np.int64)
    return _plan(dst_idx, tok, half, att_flat)


def kernel(src_emb, e_att, src_idx, dst_idx):
    from concourse.bass_utils import run_bass_kernel_spmd

    src_emb = np.asarray(src_emb, dtype=np.float32)
    pl = plan_and_build(src_idx, dst_idx, e_att)

    key = (pl["sched"],)
    if key not in _cache:
        _cache.clear()
        _cache[key] = _build_nc2(pl["sched"], pl["csum"], pl["n_tiles"], pl["C"])
    nc = _cache[key]

    embP = np.zeros((NPAIR * 2, D), dtype=np.float16)
    embP[:N_SRC] = src_emb.astype(np.float16)
    embP = np.ascontiguousarray(embP.reshape(NPAIR, P))

    in_maps = []
    for c in range(N_CORES):
        idx_flat = pl["idx3"][c].reshape(-1)  # [C*P] in (slot, lane) order
        # expand att to per-element fp16 [lane, slot*128] (contiguous multiply)
        attx = np.ascontiguousarray(
            np.repeat(pl["att4"][c], D, axis=-1)
            .transpose(1, 0, 2)
            .reshape(P, pl["C"] * P)
        )
        in_maps.append(
            {
                "embP": embP,
                "idxT": np.ascontiguousarray(_wrap_idx(idx_flat)),
                "attX": attx,
            }
        )
    kwargs = {}
    if TRACE:
        kwargs = {"trace": True, "tmpdir": TRACE_DIR}
    res = run_bass_kernel_spmd(nc, in_maps, core_ids=list(range(N_CORES)), **kwargs)
    global LAST_EXEC_NS
    LAST_EXEC_NS = res.exec_time_ns

    out_full = np.zeros((N_DST, D), dtype=np.float32)
    node_at = pl["node_at"]  # [n_tiles, 8, P]
    for c in range(N_CORES):
        ids = node_at[:, c, :].reshape(-1)
        valid = ids >= 0
        out_full[ids[valid]] = res.results[c]["out"][valid]
    return out_full


# revision 5
# speedup vs baseline: 1.1985x; 1.0111x over previous
"""GNN message passing (src_mul_edge + segment_sum) on 8 Trainium2 cores. v5.

out[n] = sum_{e : dst[e]==n} e_att[e] * src_emb[src[e]]

Non-transpose pair-token gather (baseline-proven DMA mode, tight layout):
  * src_emb rows cast to fp16 (unpadded); consecutive row PAIRS form 256-byte
    tokens in DRAM ([25088, 128] fp16). Token ids fit int16 -> SINGLE index
    window, no lo/hi split. Edge with src row r uses half r%2 of token r//2;
    the unused half is zeroed by the att table.
  * Nodes sorted by total degree, dealt 128 at a time into tiles (lane =
    node); tile span S = max degree in the 128-node window (~1% padding).
    Slot (lane, s) = flat gather position s*128+lane; per-edge descriptors.
  * dma_gather(transpose=False) from DRAM: msg[lane, s, 0:128] = token fp16.
  * att3 [128, S_total, 2] fp16 (host-built, half-selected att or 0) loaded
    once; one broadcast multiply per chunk ([128, S, 2] -> [128, S, 2, 64]);
    strided tensor_reduce over (2S) per tile -> out [128 nodes, 64] fp32,
    one DMA per tile straight to DRAM.
  * Within a node, edges sorted by token id (HBM row locality).
"""

import numpy as np

N_SRC = 50000
N_DST = 50000
D = 64
N_CORES = 8
P = 128
NPAIR = 25088
GCHUNK = 4096             # slots per gather instruction (mult of 128)

_cache: dict = {}

TRACE = False
TRACE_DIR = None
LAST_EXEC_NS = None


def _wrap_idx(idx_flat):
    w = idx_flat.reshape(-1, 16).T
    return np.tile(w, (8, 1))


def _plan(dst_idx, tok, half, att):
    """Single layout over all edges. Returns schedule + per-core arrays."""
    deg = np.bincount(dst_idx, minlength=N_DST)
    nz = np.flatnonzero(deg)
    order = nz[np.argsort(deg[nz], kind="stable")]
    n_nz = len(order)
    npad = (-n_nz) % (P * N_CORES)
    node_seq = np.concatenate([np.full(npad, -1, dtype=np.int64), order])
    n_tiles = len(node_seq) // (P * N_CORES)
    # tile t, core c, lane l -> node_seq[((t*NC)+c)*P + l]
    node_at = node_seq.reshape(n_tiles, N_CORES, P)
    degs = np.where(node_at >= 0, deg[np.clip(node_at, 0, None)], 0)
    S = degs.max(axis=(1, 2)).astype(np.int64)  # per-tile span
    S = np.maximum(S, 1)

    csum = np.concatenate([[0], np.cumsum(S)])
    C = int(csum[-1])  # total slots per lane... columns = slots per lane

    # chunks of tiles with <= budget slots (small priming chunks first)
    chunks = []  # (tile0, ntiles, [S...])
    t0 = 0
    while t0 < n_tiles:
        budget = 1024 if len(chunks) < 4 else GCHUNK
        t1 = t0
        acc = 0
        while t1 < n_tiles and (acc + S[t1]) * P <= budget:
            acc += S[t1]
            t1 += 1
        t1 = max(t1, t0 + 1)
        chunks.append((t0, t1 - t0, tuple(int(x) for x in S[t0:t1])))
        t0 = t1

    # per-edge placement
    ord_of = np.full(N_DST, -1, dtype=np.int64)
    core_of = np.full(N_DST, -1, dtype=np.int64)
    lane_of = np.full(N_DST, -1, dtype=np.int64)
    valid = node_at >= 0
    t_idx = np.broadcast_to(np.arange(n_tiles)[:, None, None], node_at.shape)
    c_idx = np.broadcast_to(np.arange(N_CORES)[None, :, None], node_at.shape)
    l_idx = np.broadcast_to(np.arange(P)[None, None, :], node_at.shape)
    ord_of[node_at[valid]] = t_idx[valid]
    core_of[node_at[valid]] = c_idx[valid]
    lane_of[node_at[valid]] = l_idx[valid]

    eorder = np.lexsort((tok, dst_idx))
    d_sorted = dst_idx[eorder]
    starts = np.concatenate([[0], np.cumsum(deg)])
    rank_e = np.arange(len(dst_idx)) - starts[d_sorted]

    t_e = ord_of[d_sorted]
    c_e = core_of[d_sorted]
    l_e = lane_of[d_sorted]
    s_e = csum[t_e] + rank_e  # slot row within the lane

    idx3 = np.zeros((N_CORES, C, P), dtype=np.int16)
    att4 = np.zeros((N_CORES, C, P, 2), dtype=np.float16)
    idx3[c_e, s_e, l_e] = tok[eorder]
    att4[c_e, s_e, l_e, half[eorder]] = att[eorder]

    return {
        "sched": tuple(chunks),
        "S": tuple(int(x) for x in S),
        "csum": csum,
        "n_tiles": n_tiles,
        "C": C,
        "idx3": idx3,
        "att4": att4,
        "node_at": node_at,
    }


def _build_nc2(sched, csum_list, n_tiles, C):
    import concourse.bacc as bacc
    import concourse.mybir as mybir
    from concourse.tile import TileContext
    from concourse.library_config import mlp

    nc = bacc.Bacc(
        "TRN2", target_bir_lowering=False, debug=False, num_swdge_queues=4
    )
    embP = nc.dram_tensor("embP", [NPAIR, P], mybir.dt.float16, kind="ExternalInput")
    idxT = nc.dram_tensor("idxT", [P, C * P // 16], mybir.dt.int16, kind="ExternalInput")
    attX = nc.dram_tensor("attX", [P, C * P], mybir.dt.float16, kind="ExternalInput")
    out = nc.dram_tensor("out", [n_tiles * P, D], mybir.dt.float32, kind="ExternalOutput")

    with TileContext(nc) as tc:
        nc.gpsimd.load_library(mlp)
        with (
            tc.tile_pool(name="tbl", bufs=1) as tbl,
            tc.tile_pool(name="msg", bufs=8) as msgp,
            tc.tile_pool(name="attx", bufs=3) as attxp,
            tc.tile_pool(name="stg", bufs=3) as stgp,
        ):
            # two-stage idx load: a small head slice unblocks the first
            # gathers while the bulk loads behind it
            head_chunks = min(6, len(sched))
            head_slots = sum(sum(Ss) for _, _, Ss in sched[:head_chunks])
            head_cols = head_slots * P // 16
            idx_a = tbl.tile([P, head_cols], mybir.dt.int16, tag="idxa")
            idx_b = tbl.tile([P, C * P // 16 - head_cols], mybir.dt.int16, tag="idxb")
            nc.sync.dma_start(idx_a[:], idxT[:, :head_cols])
            nc.sync.dma_start(idx_b[:], idxT[:, head_cols:])

            smax = max(sum(Ss) for _, _, Ss in sched)
            # greedy queue balance by descriptor count
            qload = [0, 0, 0, 0]
            for t0, ntl, Ss in sched:
                ssum = sum(Ss)
                nidx = ssum * P
                col0 = csum_list[t0]  # slot offset
                q = min(range(4), key=lambda i: qload[i])
                qload[q] += nidx
                c_lo = col0 * P // 16
                c_hi = (col0 + ssum) * P // 16
                if c_hi <= head_cols:
                    iap = idx_a[:, c_lo:c_hi]
                else:
                    iap = idx_b[:, c_lo - head_cols : c_hi - head_cols]
                msg = msgp.tile([P, smax, P], mybir.dt.float16, tag="m")
                nc.gpsimd.dma_gather(
                    msg[:, :ssum, :], embP[:, :],
                    iap,
                    nidx, nidx, P,
                    transpose=False, single_packet=False, queue_num=q,
                )
                att_x = attxp.tile([P, smax, P], mybir.dt.float16, tag="ax")
                nc.scalar.dma_start(
                    att_x[:, :ssum, :],
                    attX[:, col0 * P : (col0 + ssum) * P]
                    .rearrange("p (s d) -> p s d", d=P),
                )
                nc.vector.tensor_tensor(
                    msg[:, :ssum, :],
                    msg[:, :ssum, :],
                    att_x[:, :ssum, :],
                    mybir.AluOpType.mult,
                )
                so = 0
                for k, S in enumerate(Ss):
                    # pairwise tree over slots with contiguous fp16 adds
                    n = S
                    while n > 1:
                        h_n = n // 2
                        nc.vector.tensor_tensor(
                            msg[:, so : so + h_n, :],
                            msg[:, so : so + h_n, :],
                            msg[:, so + n - h_n : so + n, :],
                            mybir.AluOpType.add,
                        )
                        n = n - h_n
                    stage = stgp.tile([P, D], mybir.dt.float32, tag="st")
                    nc.vector.tensor_tensor(
                        stage[:, :].unsqueeze(1),
                        msg[:, so : so + 1, 0:D],
                        msg[:, so : so + 1, D : 2 * D],
                        mybir.AluOpType.add,
                    )
                    nc.sync.dma_start(
                        out[(t0 + k) * P : (t0 + k + 1) * P, :], stage[:, :]
                    )
                    so += S
    nc.compile()
    return nc


def plan_and_build(src_idx, dst_idx, e_att):
    src_idx = np.asarray(src_idx, dtype=np.int64)
    dst_idx = np.asarray(dst_idx, dtype=np.int64)
    att_flat = np.asarray(e_att, dtype=np.float16).reshape(-1)
    tok = (src_idx // 2).astype(np.int16)
    half = (src_idx & 1).astype(np.int64)
    return _plan(dst_idx, tok, half, att_flat)


def kernel(src_emb, e_att, src_idx, dst_idx):
    from concourse.bass_utils import run_bass_kernel_spmd

    src_emb = np.asarray(src_emb, dtype=np.float32)
    pl = plan_and_build(src_idx, dst_idx, e_att)

    key = (pl["sched"],)
    if key not in _cache:
        _cache.clear()
        _cache[key] = _build_nc2(pl["sched"], pl["csum"], pl["n_tiles"], pl["C"])
    nc = _cache[key]

    embP = np.zeros((NPAIR * 2, D), dtype=np.float16)
    embP[:N_SRC] = src_emb.astype(np.float16)
    embP = np.ascontiguousarray(embP.reshape(NPAIR, P))

    in_maps = []
    for c in range(N_CORES):
        idx_flat = pl["idx3"][c].reshape(-1)  # [C*P] in (slot, lane) order
        # expand att to per-element fp16 [lane, slot*128] (contiguous multiply)
        attx = np.ascontiguousarray(
            np.repeat(pl["att4"][c], D, axis=-1)
            .transpose(1, 0, 2)
            .reshape(P, pl["C"] * P)
        )
        in_maps.append(
            {
                "embP": embP,
                "idxT": np.ascontiguousarray(_wrap_idx(idx_flat)),
                "attX": attx,
            }
        )
    kwargs = {}
    if TRACE:
        kwargs = {"trace": True, "tmpdir": TRACE_DIR}
    res = run_bass_kernel_spmd(nc, in_maps, core_ids=list(range(N_CORES)), **kwargs)
    global LAST_EXEC_NS
    LAST_EXEC_NS = res.exec_time_ns

    out_full = np.zeros((N_DST, D), dtype=np.float32)
    node_at = pl["node_at"]  # [n_tiles, 8, P]
    for c in range(N_CORES):
        ids = node_at[:, c, :].reshape(-1)
        valid = ids >= 0
        out_full[ids[valid]] = res.results[c]["out"][valid]
    return out_full
